# revision 45
# baseline (speedup 1.0000x reference)
"""AUCMaxLoss (pairwise hinge over pos/neg score pairs) on 8 trn2 NeuronCores.

Algorithm: instead of the O(B^2) dense pair matrix, quantize unified scores
(u = s for pos samples, s + margin for neg samples) onto a fine 16384-bin
grid. For bins kp < kn the hinge is exactly (t - s); for equal bins the
linear half-term is exact and the dropped |t-s| residual is bounded by the
bin width (~8e-4), giving ~1e-7 relative error overall.

Per core (2048 elements): build one-hot matrices and accumulate
  cnt2[lo, hi2] = sum_e 1[lo_e=lo] * 1[hi2_e=hi2]        (PE matmul)
  w2[lo, hi2]   = sum_e u_e * 1[lo_e=lo] * 1[hi2_e=hi2]  (PE matmul)
where idx2 = idx + 16384*is_pos (doubled grid separates pos/neg halves),
hi2 = idx2 >> 7, lo = idx2 & 127. AllReduce the [128, 512] histogram block,
then every core computes
  loss_sum = sum_k  wL[k] * (CP(<k) + C[k]/2)  -  cL[k] * (SP(<k) + D[k]/2)
with C = pos counts, D = pos score sums, wL/cL the neg-half t-sum/count
tiles, and the prefixes computed with strict-upper-triangular matmuls.
"""

import os
import sys

for _p in ("/opt/trn_rl_repo", "/root/.axon_site/_ro/trn_rl_repo"):
    if os.path.isdir(_p) and _p not in sys.path:
        sys.path.insert(0, _p)

import numpy as np

import concourse.bass as bass
import concourse.tile as tile
from concourse import mybir
from concourse.bass_utils import run_bass_kernel_spmd

def _split_excess_waits(bir_json):
    """walrus in this toolchain accepts a single attached sync wait per
    compute instruction (2 for EventSemaphore, Drain can hold many), but
    Tile's sem-assignment occasionally attaches 2. Hoist the waits of any
    over-budget instruction onto a same-engine Drain inserted before it."""
    import json

    data = json.loads(bir_json)
    changed = False
    for fn in data.get("functions", []):
        for bb in fn.get("blocks", []):
            out = []
            for inst in bb.get("instructions", []):
                op = inst.get("opcode")
                eng = inst.get("engine")
                waits = (inst.get("sync_info") or {}).get("on_wait") or []
                cap = 2 if op == "EventSemaphore" else 1
                if len(waits) > cap:
                    for j, w in enumerate(waits[: len(waits) - cap]):
                        out.append(
                            {
                                "debug": inst.get("debug", 0),
                                "engine": eng,
                                "ins": [],
                                "is_reset_sema": False,
                                "name": f"{inst['name']}-wsplit{j}",
                                "opcode": "Drain",
                                "outs": [],
                                "sync_info": {"on_update": [], "on_wait": [w]},
                            }
                        )
                    inst["sync_info"]["on_wait"] = waits[len(waits) - cap :]
                    changed = True
                out.append(inst)
            bb["instructions"] = out
    if not changed:
        return bir_json
    return json.dumps(data).encode()


def _install_compile_patch():
    import concourse.bass_utils as bu

    if getattr(bu, "_wsplit_patched", False):
        return
    orig = bu.compile_bir_kernel

    def patched(bir_json, *a, **kw):
        return orig(_split_excess_waits(bir_json), *a, **kw)

    bu.compile_bir_kernel = patched
    bu._wsplit_patched = True
    try:
        from concourse import bass2jax

        bass2jax.compile_bir_kernel = patched
    except Exception:
        pass


_install_compile_patch()

N_CORES = 8
B = 16384              # batch size (fixed by the problem)
PER = B // N_CORES     # 2048 elements per core
P = 128                # SBUF partitions
F = PER // P           # 16 chunks (one free column each)
NB = 16384             # histogram bins = 128 lo x 128 hi
RLO, RHI = -6.0, 7.0   # grid range; u in [-4.0, 4.7] for these inputs
SCALE = float((NB - 1) / (RHI - RLO))
OFF = float(-RLO * SCALE)
MARGIN = 1.0
EPS = 1e-8

f32 = mybir.dt.float32
f16 = mybir.dt.float16
i32 = mybir.dt.int32
OP = mybir.AluOpType
PACK = 513  # [cL | wL | px_c | px_d | npos] columns in the AllReduce block


def _body(ctx, tc, logits, targets, out, cc_in, cc_out):
    nc = tc.nc
    const = ctx.enter_context(tc.tile_pool(name="const", bufs=1))
    prep = ctx.enter_context(tc.tile_pool(name="prep", bufs=1))
    # bufs == number of chunks: no slot recycling, so no cross-engine release
    # waits land on the TensorScalarPtr one-hot builds (walrus allows only a
    # single attached sync wait on the TS struct).
    oh = ctx.enter_context(tc.tile_pool(name="oh", bufs=F))
    big = ctx.enter_context(tc.tile_pool(name="big", bufs=1))
    ps_h = ctx.enter_context(tc.tile_pool(name="ps_h", bufs=1, space="PSUM"))
    ps_t = ctx.enter_context(tc.tile_pool(name="ps_t", bufs=1, space="PSUM"))
    ps_p = ctx.enter_context(tc.tile_pool(name="ps_p", bufs=2, space="PSUM"))

    # ---------------- constants ----------------
    iota_lo = const.tile([P, 128], i32)       # each row = 0..127
    nc.gpsimd.iota(iota_lo, pattern=[[1, 128]], base=0, channel_multiplier=0)
    iota_hi2 = const.tile([P, 256], i32)      # each row = 0..255
    nc.gpsimd.iota(iota_hi2, pattern=[[1, 256]], base=0, channel_multiplier=0)
    iota_p = const.tile([P, 1], i32)          # partition index
    nc.gpsimd.iota(iota_p, pattern=[[1, 1]], base=0, channel_multiplier=1)

    iota_lo_f = const.tile([P, 128], f32)
    nc.vector.tensor_copy(iota_lo_f, iota_lo)
    iota_hi2_f = const.tile([P, 256], f32)
    nc.vector.tensor_copy(iota_hi2_f, iota_hi2)
    iota_pf = const.tile([P, 1], f32)
    nc.vector.tensor_copy(iota_pf, iota_p)

    u_strict = const.tile([P, 128], f32)      # [p, q] = 1.0 if p < q
    nc.vector.tensor_scalar(u_strict, iota_lo_f, iota_pf, None, OP.is_gt)
    half_ident = const.tile([P, 128], f32)    # 0.5 * I
    nc.vector.tensor_scalar(half_ident, iota_lo_f, iota_pf, 0.5, OP.is_equal, OP.mult)
    ident = const.tile([P, 128], f32)         # I (for PE transpose)
    nc.vector.tensor_scalar(ident, iota_lo_f, iota_pf, None, OP.is_equal)
    ones_col = const.tile([P, 1], f32)
    nc.vector.memset(ones_col, 1.0)
    ones_row = const.tile([1, 128], f32)
    nc.vector.memset(ones_row, 1.0)



    # bucket-center-per-bin constant, built off the DVE critical path:
    # the iota wraps at 16384 via base_grid_id = (128*hi2 + lo) mod 16384,
    # and ACT applies the int->float convert and the affine map.
    ci = const.tile([P, 256], i32)            # global bin id: 128*hi2 + lo
    nc.gpsimd.iota(ci, pattern=[[128, 256]], base=0, channel_multiplier=1)
    nc.vector.tensor_scalar(ci, ci, 16383, None, OP.bitwise_and)
    cif = const.tile([P, 256], f32)
    nc.scalar.copy(cif, ci)
    cbias = const.tile([P, 1], f32)
    nc.vector.memset(cbias, -OFF / SCALE)
    center_t = const.tile([P, 256], f32)      # bucket center value per bin
    nc.scalar.activation(
        center_t, cif, mybir.ActivationFunctionType.Identity,
        bias=cbias, scale=1.0 / SCALE,
    )

    # ---------------- load inputs ----------------
    lg = prep.tile([P, F, 2], f32)
    nc.sync.dma_start(out=lg, in_=logits.rearrange("(p f) c -> p f c", p=P))
    tg = prep.tile([P, F], i32)
    nc.sync.dma_start(out=tg, in_=targets.rearrange("(p f) -> p f", p=P))

    # ---------------- per-element prep ----------------
    tgf = prep.tile([P, F], f32)
    nc.vector.tensor_copy(tgf, tg)
    m = prep.tile([P, F], f32)                # pos mask
    nc.vector.tensor_scalar(m, tgf, 1.0, None, OP.is_equal)
    d = prep.tile([P, F], f32)
    nc.vector.tensor_tensor(d, lg[:, :, 1], lg[:, :, 0], OP.subtract)
    md = prep.tile([P, F], f32)
    nc.vector.tensor_tensor(md, m, d, OP.mult)
    s = prep.tile([P, F], f32)                # score of true class
    nc.vector.tensor_tensor(s, lg[:, :, 0], md, OP.add)
    u = prep.tile([P, F], f32)                # pos -> s, neg -> s + MARGIN
    nc.vector.tensor_scalar(u, m, -MARGIN, MARGIN, OP.mult, OP.add)
    nc.vector.tensor_tensor(u, s, u, OP.add)
    y = prep.tile([P, F], f32)                # clamped grid coordinate
    nc.vector.tensor_scalar(y, u, SCALE, OFF, OP.mult, OP.add)
    nc.vector.tensor_scalar(y, y, 0.0, float(NB - 1), OP.max, OP.min)
    m16 = prep.tile([P, F], f32)
    nc.vector.tensor_scalar(m16, m, float(NB), None, OP.mult)
    y2 = prep.tile([P, F], f32)               # doubled grid: pos half at +NB
    nc.vector.tensor_tensor(y2, y, m16, OP.add)
    idx2 = prep.tile([P, F], i32)
    nc.vector.tensor_copy(idx2, y2)           # f32 -> i32 (any monotone rounding ok)
    hi2 = prep.tile([P, F], i32)
    nc.vector.tensor_scalar(hi2, idx2, 7, None, OP.arith_shift_right)
    lo = prep.tile([P, F], i32)
    nc.vector.tensor_scalar(lo, idx2, 127, None, OP.bitwise_and)
    hi2f = prep.tile([P, F], f32)
    nc.vector.tensor_copy(hi2f, hi2)
    lof = prep.tile([P, F], f32)
    nc.vector.tensor_copy(lof, lo)
    idx = prep.tile([P, F], i32)              # base-grid index (pos bit stripped)
    nc.vector.tensor_scalar(idx, idx2, 16383, None, OP.bitwise_and)
    idxf = prep.tile([P, F], f32)
    nc.vector.tensor_copy(idxf, idx)
    r32 = prep.tile([P, F], f32)              # residual u - bucket_center
    nc.vector.tensor_scalar(r32, idxf, 1.0 / SCALE, -OFF / SCALE, OP.mult, OP.add)
    nc.vector.tensor_tensor(r32, u, r32, OP.subtract)

    # ---------------- histogram accumulation (fp16 one-hots) ----------------
    # cnt2[lo, hi2] counts; r2[lo, hi2] sums the small residual
    # r = u - bucket_center (|r| <= bin_width/2 ~ 4e-4, exact enough in fp16).
    # w2 = cnt2 * center + r2 is reconstructed after the loop.
    cnt2_ps = ps_h.tile([P, 256], f32, tag="cnt2")
    r2_ps = ps_h.tile([P, 256], f32, tag="r2")
    for c in range(F):
        h_lo = oh.tile([P, 128], f16, tag="h_lo")
        nc.vector.tensor_scalar(h_lo, iota_lo_f, lof[:, c : c + 1], None, OP.is_equal)
        h_hi = oh.tile([P, 256], f16, tag="h_hi")
        nc.vector.tensor_scalar(h_hi, iota_hi2_f, hi2f[:, c : c + 1], None, OP.is_equal)
        w_hi = oh.tile([P, 256], f16, tag="w_hi")
        if c % 2 == 0:  # alternate engines so neither paces the loop
            nc.scalar.mul(w_hi, h_hi, r32[:, c : c + 1])
        else:
            nc.vector.tensor_scalar(
                w_hi, iota_hi2_f, hi2f[:, c : c + 1], r32[:, c : c + 1],
                OP.is_equal, OP.mult,
            )
        nc.tensor.matmul(cnt2_ps, h_lo, h_hi, start=(c == 0), stop=(c == F - 1))
        nc.tensor.matmul(r2_ps, h_lo, w_hi, start=(c == 0), stop=(c == F - 1))

    # ---------------- local linear stage: w2, prefix tiles, npos ----------------
    cnt2_sb = big.tile([P, 256], f32, tag="cnt2_sb")
    nc.scalar.copy(cnt2_sb, cnt2_ps)
    w2_sb = big.tile([P, 256], f32, tag="w2_sb")
    nc.vector.tensor_tensor(w2_sb, cnt2_sb, center_t, OP.mult)
    nc.vector.tensor_tensor(w2_sb, w2_sb, r2_ps, OP.add)

    cR = cnt2_sb[:, 128:256]   # pos counts (C)
    wR = w2_sb[:, 128:256]     # pos s-sums (D)

    # PX = strict_prefix(X) + X/2 over global bin order; linear in X, so it
    # commutes with the AllReduce sum and can be computed on local partials.
    px_list = []
    for X in (cR, wR):
        xt_ps = ps_t.tile([P, 128], f32, tag="xt")
        nc.tensor.transpose(xt_ps, X, ident)
        xt_sb = big.tile([P, 128], f32, tag="xt_sb")
        nc.scalar.copy(xt_sb, xt_ps)
        w1_ps = ps_t.tile([P, 128], f32, tag="w1")
        nc.tensor.matmul(w1_ps, xt_sb, u_strict)        # [lo, hi] = sum_{hi'<hi} X[lo, hi']
        w1_sb = big.tile([P, 128], f32, tag="w1_sb")
        nc.scalar.copy(w1_sb, w1_ps)
        base_ps = ps_t.tile([1, 128], f32, tag="msc")
        nc.tensor.matmul(base_ps, ones_col, w1_sb)      # [1, hi] = sum_lo w1[lo, hi]
        base_sb = big.tile([1, 128], f32, tag="base_sb")
        nc.scalar.copy(base_sb, base_ps)
        px_ps = ps_p.tile([P, 128], f32, tag="px")
        nc.tensor.matmul(px_ps, u_strict, X, start=True, stop=False)
        nc.tensor.matmul(px_ps, ones_row, base_sb, start=False, stop=False)
        nc.tensor.matmul(px_ps, half_ident, X, start=False, stop=True)
        px_list.append(px_ps)
    px_sb = big.tile([P, 256], f32, tag="px_sb")
    nc.scalar.copy(px_sb[:, 0:128], px_list[0])
    nc.scalar.copy(px_sb[:, 128:256], px_list[1])

    redp = big.tile([P, 1], f32, tag="redp")
    nc.vector.reduce_sum(redp, cR, axis=mybir.AxisListType.X)

    # ---------------- AllReduce just the prefix tiles ----------------
    # F = sum_k wL_g[k]*PXC_g[k] - cL_g[k]*PXD_g[k] expands over cores as
    # sum_me sum_k wL_me[k]*PXC_g[k] - ..., so only PX needs to be global;
    # each core keeps its local wL/cL half and emits a partial dot.
    nc.sync.dma_start(out=cc_in[:], in_=px_sb)
    nc.gpsimd.collective_compute(
        "AllReduce",
        OP.add,
        replica_groups=[list(range(N_CORES))],
        ins=[cc_in[:]],
        outs=[cc_out[:]],
    )
    g = big.tile([P, 256], f32, tag="g_sb")
    nc.sync.dma_start(out=g, in_=cc_out[:])

    # ---------------- partial bilinear dot ----------------
    # scalar_tensor_tensor with accum_out fuses multiply + row-reduction;
    # host combines as loss_sum = sum(col0) - sum(col1), n_pos = sum(col2).
    trash = big.tile([P, 128], f32, tag="trash")
    red = big.tile([P, 3], f32, tag="red")
    nc.vector.scalar_tensor_tensor(
        out=trash, in0=w2_sb[:, 0:128], scalar=1.0, in1=g[:, 0:128],
        op0=OP.bypass, op1=OP.mult, accum_out=red[:, 0:1],
    )
    nc.vector.scalar_tensor_tensor(
        out=trash, in0=cnt2_sb[:, 0:128], scalar=1.0, in1=g[:, 128:256],
        op0=OP.bypass, op1=OP.mult, accum_out=red[:, 1:2],
    )
    nc.vector.tensor_copy(red[:, 2:3], redp)
    tot_ps = ps_t.tile([1, 3], f32, tag="msc")
    nc.tensor.matmul(tot_ps, ones_col, red)         # [1,3] partial sums
    tot_sb = big.tile([1, 3], f32, tag="tot_sb")
    nc.scalar.copy(tot_sb, tot_ps)
    nc.sync.dma_start(out=out[:], in_=tot_sb)


def build_nc():
    nc = bass.Bass()
    logits = nc.declare_dram_parameter("logits", [PER, 2], f32, isOutput=False)
    targets = nc.declare_dram_parameter("targets", [PER], i32, isOutput=False)
    out = nc.declare_dram_parameter("out", [1, 3], f32, isOutput=True)
    cc_in = nc.dram_tensor("cc_in", [P, 256], f32)
    cc_out = nc.dram_tensor("cc_out", [P, 256], f32, addr_space="Shared")
    from contextlib import ExitStack

    with tile.TileContext(nc) as tc:
        with ExitStack() as ctx:
            _body(ctx, tc, logits, targets, out, cc_in, cc_out)
    return nc


_NC_CACHE = {}


def _get_nc():
    if "nc" not in _NC_CACHE:
        _NC_CACHE["nc"] = build_nc()
    return _NC_CACHE["nc"]


def _in_maps(inputs):
    logits = np.ascontiguousarray(np.asarray(inputs["logits"], dtype=np.float32))
    targets = np.asarray(inputs["targets"]).astype(np.int32)
    assert logits.shape == (B, 2) and targets.shape == (B,)
    maps = []
    for c in range(N_CORES):
        sl = slice(c * PER, (c + 1) * PER)
        maps.append(
            {
                "logits": np.ascontiguousarray(logits[sl]),
                "targets": np.ascontiguousarray(targets[sl]),
            }
        )
    return maps


def _ensure_ntff_hook():
    """The image's antenv package lacks axon_hooks; synthesize it so
    run_bass_kernel_spmd(trace=True) can reach the axon NTFF profiler."""
    import types

    try:
        import antenv
        from antenv import axon_hooks  # noqa: F401

        return
    except ImportError:
        pass
    try:
        import antenv

        mod = types.ModuleType("antenv.axon_hooks")
        _hook = [None]
        mod.set_axon_ntff_profile_hook = lambda h: _hook.__setitem__(0, h)
        mod.get_axon_ntff_profile_hook = lambda: _hook[0]
        sys.modules["antenv.axon_hooks"] = mod
        antenv.axon_hooks = mod
        from trn_agent_boot.trn_boot import _ntff_profile_via_ctypes

        mod.set_axon_ntff_profile_hook(
            _ntff_profile_via_ctypes("/opt/axon/libaxon_pjrt.so")
        )
    except Exception as e:  # degrade: tracing skipped, run still works
        print(f"[ntff-hook] install failed: {e}", file=sys.stderr)


def _run(inputs, trace=False, trace_cores=None):
    if trace:
        _ensure_ntff_hook()
    nc = _get_nc()
    res = run_bass_kernel_spmd(
        nc,
        _in_maps(inputs),
        core_ids=list(range(N_CORES)),
        trace=trace,
        trace_cores=trace_cores,
    )
    return res


def combine(parts):
    """Host-side unshard: psum the per-core partials [pos_dot, neg_dot, n_pos]."""
    parts = np.asarray(parts, dtype=np.float32).reshape(N_CORES, 3)
    loss_sum = np.float32(parts[:, 0].sum(dtype=np.float32)) - np.float32(
        parts[:, 1].sum(dtype=np.float32)
    )
    n_pos = np.float32(parts[:, 2].sum(dtype=np.float32))
    n_pairs = n_pos * np.float32(B - n_pos)
    return np.float32(loss_sum / (n_pairs + np.float32(EPS)))


def kernel(**inputs) -> np.ndarray:
    res = _run(inputs)
    return combine([res.results[c]["out"] for c in range(N_CORES)])


if __name__ == "__main__":
    rng = np.random.default_rng(0)
    logits = rng.standard_normal((B, 2), dtype=np.float32)
    targets = rng.integers(0, 2, size=B).astype(np.int64)
    print("loss:", kernel(logits=logits, targets=targets))
